# revision 30
# baseline (speedup 1.0000x reference)
"""Bilinear attention (a = causal(a1*a2), no softmax) on 8 Trainium2 cores.

Sharding: core = (batch, key-parity). Each core processes all queries of its
batch against the even- or odd-indexed 128-key blocks. With causal masking,
query-chunk c (512 queries) needs exactly 2c+2 parity-strip key blocks on
either parity, so a single SPMD program covers both cores of a pair; the
per-core difference lives entirely in host-side data layout. Partial outputs
(each pair member saw half the keys) are summed on host.

v4 (bf16 + pipelining): all matmul inputs are bfloat16 (PE at 1.0 cyc/row
for any free dim, input DMA halved, SBUF traffic halved). PSUM stays fp32;
the fp32->bf16 casts ride the PSUM->SBUF copies. The RoPE rotation is a
partition-half swap done with two SBUF->SBUF DMAs (negation folded into a
host-side [-sin; sin] table), freeing the PE of all rotation matmuls; the
rope multiplies/adds are pure-bf16 on DVE/GpSimd. The strip loop is
software-pipelined with a 2-strip lookahead (5 score PSUM banks + 1 out
bank): out(i) is issued on PE after the score matmuls of strip i+2, hiding
the Act-copy + DVE-mul combine chain (~1.5us) behind two strips of PE
work. o_ps uses a single PSUM accumulation group (start on the first
piece, bank-wide pending-zero consumed lazily). The Wo/y epilogue of chunk
c is a generator interleaved into chunk c+1's strip loop after the
boundary generator is exhausted (fills the last chunk's strip stalls,
shortens the tail).

Device layout is transpose-free: the host ships x^T per query chunk with the
4 query blocks permuted [4c+p, 4c+2+p, 4c+3-p, 4c+1-p] (parity-first, then
descending), so the kv projections read the first 256 columns of the same
tile the q projections read. Projections produce q^T/k^T [d, s] tiles;
scores are computed transposed (a^T[t, sq] = matmul(lhsT=k^T block,
rhs=q^T)); v comes out of a PE transpose as [t, d]; out^T[d, sq]
accumulates in PSUM over key blocks and feeds the Wo matmul as stationary.
"""

import sys

if "/opt/trn_rl_repo" not in sys.path:
    sys.path.insert(0, "/opt/trn_rl_repo")

import numpy as np
import ml_dtypes

import concourse.bass as bass
import concourse.mybir as mybir
import concourse.tile as tile
from concourse import bacc
from concourse.bass_utils import run_bass_kernel_spmd

B, S, D, DH = 4, 4096, 1024, 128
NCH = 8          # query chunks per batch
SQ = S // NCH    # 512 queries per chunk
TB = 128         # key block
NSTRIP = S // TB // 2  # 16 strip blocks per core
DC = D // 128    # 8 contraction chunks
F32 = mybir.dt.float32
F32R = mybir.dt.float32r
BF16 = mybir.dt.bfloat16

# wimg column offsets: k1 k2 v q1 q2 (each DC*128 cols) then Wo (D cols)
WOFF = {name: i * D for i, name in enumerate(["k1", "k2", "v", "q1", "q2"])}
WO_OFF = 5 * D
WIMG_COLS = 5 * D + D

_compiled = {}


def _edge(ap):
    """Strided view of a [128, 512] AP selecting cols {0:128} u {384:512}
    as a [128, 2, 128] access pattern."""
    a = ap[:, 0:128]
    return bass.AP(tensor=a.tensor, offset=a.offset,
                   ap=[a.ap[0], [384, 2], [1, 128]])


def _swap(nc, rot, src, n):
    """rot = partition-half swap of src ([x1;x2] -> [x2;x1]) via two
    SBUF->SBUF DMAs (issue cost split across two queues). The rope negation
    is folded into the host-side sin table ([-sin; sin])."""
    nc.gpsimd.dma_start(rot[0:64, :n], src[64:128, :n])
    nc.sync.dma_start(rot[64:128, :n], src[0:64, :n])


def _rope(nc, tmp, rsp, dst, src_sbuf, cs2, n):
    """dst(BF16) = rope(src) in [d, s] layout.

    rot = partition-swapped src (DMA), then
    dst = src*[cos;cos] + rot*[-sin;sin]. cs2: [128, 2*SQ] BF16 SBUF with
    [cos;cos] in cols 0:SQ and [-sin;sin] in cols SQ:2*SQ. src_sbuf BF16.
    Pure-bf16 muls on DVE, pure-bf16 add on GpSimd.
    """
    rot = rsp.tile([128, SQ], BF16, tag="rot")
    _swap(nc, rot, src_sbuf, n)
    m = tmp.tile([128, n], BF16, tag="ropetmp")
    nc.vector.tensor_mul(m[:], src_sbuf[:, :n], cs2[:, 0:n])
    t = tmp.tile([128, n], BF16, tag="ropetmp")
    nc.vector.tensor_mul(t[:], rot[:, :n], cs2[:, SQ : SQ + n])
    nc.vector.tensor_add(dst[:, :n], m[:], t[:])


def _rope2(nc, tmp, rsp, dst, src_sbuf, cs2):
    """Rope a [128, 512] (k1|k2) pair into BF16; both halves use kv-col
    cos/sin."""
    rot = rsp.tile([128, SQ], BF16, tag="rot")
    _swap(nc, rot, src_sbuf, SQ)
    # cs broadcast: repeat cs2[:, 0:256] (cc) / cs2[:, SQ:SQ+256] (ssn) twice
    def rep2(base_off):
        a = cs2[:, base_off : base_off + 256]
        return bass.AP(tensor=a.tensor, offset=a.offset,
                       ap=[a.ap[0], [0, 2], [1, 256]])
    m = tmp.tile([128, 512], BF16, tag="ropetmp")
    nc.vector.tensor_mul(m[:].rearrange("p (r n) -> p r n", r=2), src_sbuf[:].rearrange("p (r n) -> p r n", r=2), rep2(0))
    t = tmp.tile([128, 512], BF16, tag="ropetmp")
    nc.vector.tensor_mul(t[:].rearrange("p (r n) -> p r n", r=2), rot[:].rearrange("p (r n) -> p r n", r=2), rep2(SQ))
    nc.vector.tensor_add(dst[:], m[:], t[:])


def _proj(nc, psum, wimg, name, xq_t, ncols, start_col=0, psum_col=0):
    """Accumulate a [128, ncols] projection into psum over the DC chunks."""
    off = WOFF[name]
    for j in range(DC):
        nc.tensor.matmul(
            psum[:, psum_col : psum_col + ncols],
            wimg[:, off + j * 128 : off + (j + 1) * 128],
            xq_t[:, j, start_col : start_col + ncols],
            start=(j == 0),
            stop=(j == DC - 1),
        )


def _build(variant):
    """variant: 'causal' | 'dense' | 'generic'."""
    assert variant in ("causal", "dense", "generic")
    nc = bacc.Bacc("TRN2", target_bir_lowering=False, debug=False, num_devices=8)

    xq = nc.dram_tensor("xq", [NCH, 128, DC * SQ], BF16, kind="ExternalInput")
    cs = nc.dram_tensor("cs", [NCH, 128, 2 * SQ], BF16, kind="ExternalInput")
    wimg = nc.dram_tensor("wimg", [128, WIMG_COLS], BF16, kind="ExternalInput")
    iden = nc.dram_tensor("iden", [128, 128], F32R, kind="ExternalInput")
    if variant == "causal":
        # dmask: [128, 256] = [tri-keep | X] (X = ones on even cores, zeros
        # on odd); serves both the strided diag edge {0:128}u{384:512} and
        # the half-width sub strip cols 128:384.
        dmask = nc.dram_tensor("dmask", [128, 256], F32, kind="ExternalInput")
        # wide mask for chunk 0's sub strip (runs full width to own the
        # PSUM accumulation start): [0 | tri | X | 0]
        dmaskw = nc.dram_tensor("dmaskw", [128, SQ], F32, kind="ExternalInput")
    if variant == "generic":
        gmask = nc.dram_tensor(
            "gmask", [NCH, NSTRIP, 128, SQ], F32, kind="ExternalInput"
        )
    y = nc.dram_tensor("y", [NCH, 128, (SQ // 128) * D], BF16, kind="ExternalOutput")

    n_iter = (lambda c: 2 * c + 2) if variant == "causal" else (lambda c: NSTRIP)

    with tile.TileContext(nc) as tc:
        with (
            tc.tile_pool(name="consts", bufs=1) as consts,
            tc.tile_pool(name="kv", bufs=1) as kvpool,
            tc.tile_pool(name="xqp", bufs=3) as xqp,
            tc.tile_pool(name="csp", bufs=3) as csp,
            tc.tile_pool(name="qp", bufs=2) as qp,
            tc.tile_pool(name="ropetmp", bufs=6) as pool64,
            tc.tile_pool(name="ropesrc", bufs=4) as ropesrc,
            tc.tile_pool(name="rsp", bufs=3) as rsp,
            tc.tile_pool(name="ap", bufs=6) as apool,
            tc.tile_pool(name="s1p", bufs=6) as s1p,
            tc.tile_pool(name="osb", bufs=3) as osb,
            tc.tile_pool(name="ysb", bufs=3) as ysb,
            tc.tile_pool(name="gmp", bufs=3) as gmp,
            tc.tile_pool(name="sps", bufs=5, space="PSUM") as sps,
            tc.tile_pool(name="bps", bufs=2, space="PSUM") as bps,
            tc.tile_pool(name="ops", bufs=1, space="PSUM") as ops,
        ):
            wimg_t = consts.tile([128, WIMG_COLS], BF16)
            iden_t = consts.tile([128, 128], F32R)
            if variant == "causal":
                dmask_t = consts.tile([128, 256], F32)
                dmaskw_t = consts.tile([128, SQ], F32)

            def load_chunk(c):
                # kv halves (cols 0:256 of each contraction piece) first:
                # the kv projections of the boundary run a chunk before the
                # q projections need the rest. Issues alternate between the
                # sync and gpsimd queues to halve sequencer serialization.
                xq_t = xqp.tile([128, DC, SQ], BF16, tag="xq")
                src = xq.ap()[c].rearrange("p (j n) -> p j n", j=DC)
                nc.sync.dma_start(xq_t[:, 0:4, 0:256], src[:, 0:4, 0:256])
                nc.gpsimd.dma_start(xq_t[:, 4:8, 0:256], src[:, 4:8, 0:256])
                cs_t = csp.tile([128, 2 * SQ], BF16, tag="cs")
                nc.sync.dma_start(cs_t[:], cs.ap()[c])
                nc.gpsimd.dma_start(xq_t[:, 0:4, 256:512], src[:, 0:4, 256:512])
                nc.sync.dma_start(xq_t[:, 4:8, 256:512], src[:, 4:8, 256:512])
                return xq_t, cs_t

            def load_consts_ordered(first_io):
                # prologue: spread issues over four queues so the DMA-issue
                # train (~0.7us per dma_start on one sequencer) doesn't gate
                # the first projections; first pieces are the j<4 half of
                # the k1 weights and the first chunk-0 kv piece
                nc.sync.dma_start(wimg_t[:, : D // 2], wimg.ap()[:, : D // 2])
                io0 = None
                if first_io is not None:
                    xq_t = xqp.tile([128, DC, SQ], BF16, tag="xq")
                    src = xq.ap()[0].rearrange("p (j n) -> p j n", j=DC)
                    nc.gpsimd.dma_start(xq_t[:, 0:4, 0:256], src[:, 0:4, 0:256])
                nc.scalar.dma_start(wimg_t[:, D // 2 : D], wimg.ap()[:, D // 2 : D])
                if first_io is not None:
                    nc.scalar.dma_start(xq_t[:, 4:8, 0:256], src[:, 4:8, 0:256])
                    cs_t = csp.tile([128, 2 * SQ], BF16, tag="cs")
                    nc.gpsimd.dma_start(cs_t[:], cs.ap()[0])
                nc.sync.dma_start(wimg_t[:, D : 3 * D], wimg.ap()[:, D : 3 * D])
                nc.gpsimd.dma_start(wimg_t[:, 3 * D : 5 * D],
                                    wimg.ap()[:, 3 * D : 5 * D])
                if first_io is not None:
                    nc.scalar.dma_start(xq_t[:, 0:4, 256:512],
                                        src[:, 0:4, 256:512])
                    nc.gpsimd.dma_start(xq_t[:, 4:8, 256:512],
                                        src[:, 4:8, 256:512])
                    io0 = (xq_t, cs_t)
                nc.sync.dma_start(iden_t[:], iden.ap())
                if variant == "causal":
                    nc.gpsimd.dma_start(dmask_t[:], dmask.ap())
                    nc.scalar.dma_start(dmaskw_t[:], dmaskw.ap())
                nc.sync.dma_start(wimg_t[:, 5 * D :], wimg.ap()[:, 5 * D :])
                return io0

            k12a = kvpool.tile([128, NCH * 512], BF16, tag="k12a")
            va = kvpool.tile([128, 2 * NSTRIP * TB], BF16, tag="va")

            def k1_sl(i):
                return k12a[:, (i // 2) * 512 + (i % 2) * TB :][:, :TB]

            def k2_sl(i):
                return k12a[:, (i // 2) * 512 + 256 + (i % 2) * TB :][:, :TB]

            def boundary(c):
                """Generator: emits chunk-c kv/q projections in steps.
                Order: k, q1, q2, then v — the q ropes' swap-DMA latency
                hides under the v stage before chunk c's strips start."""
                xq_t, cs_t = chunk_io.pop(c)
                if variant == "causal":
                    kps = bps.tile([128, 512], F32, tag="bps")
                    _proj(nc, kps, wimg_t, "k1", xq_t, 256)
                    yield
                    _proj(nc, kps, wimg_t, "k2", xq_t, 256, psum_col=256)
                    ksb = ropesrc.tile([128, 512], BF16, tag="ropesrc")
                    nc.scalar.copy(ksb[:], kps[:])
                    yield
                    _rope2(nc, pool64, rsp, k12a[:, 512 * c : 512 * (c + 1)],
                           ksb, cs_t)
                    yield
                qs = []
                for name in ("q1", "q2"):
                    qps = bps.tile([128, 512], F32, tag="bps")
                    _proj(nc, qps, wimg_t, name, xq_t, SQ)
                    qsb = ropesrc.tile([128, SQ], BF16, tag="ropesrc")
                    nc.scalar.copy(qsb[:], qps[:])
                    yield
                    qdst = qp.tile([128, SQ], BF16, tag=f"{name}s")
                    _rope(nc, pool64, rsp, qdst, qsb, cs_t, SQ)
                    qs.append(qdst)
                    yield
                chunk_q[c] = qs
                if variant == "causal":
                    vps = bps.tile([128, 512], F32, tag="bps")
                    _proj(nc, vps, wimg_t, "v", xq_t, 256)
                    vsb = ropesrc.tile([128, 256], F32R, tag="vsb")
                    nc.scalar.copy(vsb[:], vps[:, :256])
                    yield
                    vtp = bps.tile([128, 512], F32R, tag="bps")
                    nc.tensor.transpose(vtp[:, 0:128], vsb[:, 0:128], iden_t[:])
                    nc.tensor.transpose(vtp[:, 128:256], vsb[:, 128:256], iden_t[:])
                    nc.scalar.copy(va[:, 256 * c : 256 * (c + 1)], vtp[:, 0:256])
                    yield

            def epilogue(c, o_ps):
                """Generator: Wo projection + y store for chunk c."""
                o_sb = osb.tile([128, SQ], BF16, tag="osb")
                nc.scalar.copy(o_sb[:], o_ps[:])
                yield
                y_sb = ysb.tile([128, (SQ // 128) * D], BF16, tag="ysb")
                for r in range(SQ // 128):
                    for h in range(D // 512):
                        y_ps = sps.tile([128, 512], F32, tag="sps")
                        nc.tensor.matmul(
                            y_ps[:],
                            o_sb[:, 128 * r : 128 * (r + 1)],
                            wimg_t[:, WO_OFF + 512 * h : WO_OFF + 512 * (h + 1)],
                            start=True, stop=True,
                        )
                        if (r + h) % 2 == 0:
                            nc.vector.tensor_copy(
                                y_sb[:, r * D + 512 * h : r * D + 512 * (h + 1)],
                                y_ps[:])
                        else:
                            nc.scalar.copy(
                                y_sb[:, r * D + 512 * h : r * D + 512 * (h + 1)],
                                y_ps[:])
                        yield
                    # alternate output queues so the final drain is split
                    eng = nc.gpsimd if r % 2 == 0 else nc.sync
                    eng.dma_start(
                        y.ap()[c][:, r * D : (r + 1) * D],
                        y_sb[:, r * D : (r + 1) * D])

            chunk_q = {}
            chunk_io = {}

            if variant != "causal":
                # project all kv first (dense needs late key blocks early)
                load_consts_ordered(None)
                for c in range(NCH):
                    xq_t, cs_t = load_chunk(c)
                    kps = bps.tile([128, 512], F32, tag="bps")
                    _proj(nc, kps, wimg_t, "k1", xq_t, 256)
                    _proj(nc, kps, wimg_t, "k2", xq_t, 256, psum_col=256)
                    ksb = ropesrc.tile([128, 512], BF16, tag="ropesrc")
                    nc.scalar.copy(ksb[:], kps[:])
                    _rope2(nc, pool64, rsp, k12a[:, 512 * c : 512 * (c + 1)],
                           ksb, cs_t)
                    vps = bps.tile([128, 512], F32, tag="bps")
                    _proj(nc, vps, wimg_t, "v", xq_t, 256)
                    vsb = ropesrc.tile([128, 256], F32R, tag="vsb")
                    nc.scalar.copy(vsb[:], vps[:, :256])
                    vtp = bps.tile([128, 512], F32R, tag="bps")
                    nc.tensor.transpose(vtp[:, 0:128], vsb[:, 0:128], iden_t[:])
                    nc.tensor.transpose(vtp[:, 128:256], vsb[:, 128:256], iden_t[:])
                    nc.scalar.copy(va[:, 256 * c : 256 * (c + 1)], vtp[:, 0:256])

            # prologue: load chunk-0 inputs + weights in service order,
            # then chunk 1, then run chunk-0 boundary
            chunk_io[0] = load_consts_ordered(True)
            chunk_io[1] = load_chunk(1)
            for _ in boundary(0):
                pass

            prev_epi = None  # epilogue generator of the previous chunk
            for c in range(NCH):
                if c + 2 < NCH:
                    chunk_io[c + 2] = load_chunk(c + 2)
                bw = boundary(c + 1) if c + 1 < NCH else None
                q1s, q2s = chunk_q.pop(c)
                o_ps = ops.tile([128, SQ], F32, tag="ops")
                ni = n_iter(c)

                def blk_scores(i, qcol0, qcol1):
                    a1 = sps.tile([128, SQ], F32, tag="sps")
                    nc.tensor.matmul(a1[:, qcol0:qcol1], k1_sl(i),
                                     q1s[:, qcol0:qcol1], start=True, stop=True)
                    a2 = sps.tile([128, SQ], F32, tag="sps")
                    nc.tensor.matmul(a2[:, qcol0:qcol1], k2_sl(i),
                                     q2s[:, qcol0:qcol1], start=True, stop=True)
                    return a1, a2

                def fill():
                    """Advance background work: up to two boundary steps of
                    the next chunk (so its rope chains finish well before the
                    chunk transition), else one previous-epilogue step."""
                    if bw is not None:
                        if next(bw, "done") != "done":
                            next(bw, None)
                            return
                    if prev_epi is not None:
                        next(prev_epi, None)

                if variant == "causal":
                    # One PSUM accumulation group per o_ps bank: start=True
                    # only on the first piece (bank-wide pending-zero is
                    # consumed lazily by the other pieces' first writes),
                    # stop=True only on the very last piece.
                    started = [False]
                    # pending out halves: list of (strip, a_s, [(q0,q1),...])
                    pending = []

                    def emit_out(last=False):
                        if not pending:
                            return
                        i, a_s, pieces = pending.pop(0)
                        for pi, (q0, q1v) in enumerate(pieces):
                            st = not started[0]
                            started[0] = True
                            nc.tensor.matmul(
                                o_ps[:, q0:q1v], va[:, TB * i : TB * (i + 1)],
                                a_s[:, q0:q1v], start=st,
                                stop=(last and pi == len(pieces) - 1),
                            )

                    def note(entry):
                        # 2-strip lookahead: out(i) is issued after the score
                        # matmuls of strip i+2 (5 sps banks hold 2.5 strips)
                        if len(pending) >= 2:
                            emit_out()
                        pending.append(entry)

                    def full_strip(i):
                        """Scores + combine for an unmasked strip."""
                        a1, a2 = blk_scores(i, 0, SQ)
                        a_s = apool.tile([128, SQ], BF16, tag="a")
                        t = s1p.tile([128, SQ], F32, tag="s1")
                        nc.scalar.copy(t[:], a1[:])
                        nc.vector.tensor_mul(a_s[:], t[:], a2[:])
                        return a_s, [(0, 512)]

                    def sub_strip():
                        """The half-masked strip (keys 2c+1): only query cols
                        128:384 are live."""
                        i = 2 * c + 1
                        a1, a2 = blk_scores(i, 128, 384)
                        a_s = apool.tile([128, SQ], BF16, tag="a")
                        t = s1p.tile([128, 256], F32, tag="s1h")
                        nc.vector.tensor_mul(t[:], a1[:, 128:384], dmask_t[:])
                        nc.vector.tensor_mul(a_s[:, 128:384], t[:],
                                             a2[:, 128:384])
                        return i, a_s, [(128, 384)]

                    def diag_strip():
                        """Diag strip: mask only the edge cols
                        {0:128} u {384:512}."""
                        i = 2 * c
                        a1, a2 = blk_scores(i, 0, SQ)
                        t = s1p.tile([128, SQ], F32, tag="s1d")
                        dmv = dmask_t[:].rearrange("p (r n) -> p r n", r=2)
                        nc.vector.tensor_mul(
                            t[:, 0:256].rearrange("p (r n) -> p r n", r=2),
                            _edge(a1), dmv)
                        a_s = apool.tile([128, SQ], BF16, tag="a")
                        nc.vector.tensor_mul(
                            _edge(a_s),
                            t[:, 0:256].rearrange("p (r n) -> p r n", r=2),
                            _edge(a2))
                        nc.scalar.copy(t[:, 256:512], a1[:, 128:384])
                        nc.vector.tensor_mul(a_s[:, 128:384], t[:, 256:512],
                                             a2[:, 128:384])
                        return i, a_s, [(0, 512)]

                    def causal_steps():
                        if c == 0:
                            # chunk 0's sub strip runs full width with the
                            # wide mask to own the PSUM start
                            a1, a2 = blk_scores(1, 0, SQ)
                            a_s = apool.tile([128, SQ], BF16, tag="a")
                            t = s1p.tile([128, SQ], F32, tag="s1")
                            nc.vector.tensor_mul(t[:], a1[:], dmaskw_t[:])
                            nc.vector.tensor_mul(a_s[:], t[:], a2[:])
                            note((1, a_s, [(0, 512)]))
                            yield
                            note(diag_strip())
                            yield
                        else:
                            # order: full0 (owns PSUM start), sub, diag,
                            # remaining fulls (last full has the shortest
                            # combine chain -> minimal tail before epilogue)
                            a_s, pieces = full_strip(0)
                            note((0, a_s, pieces))
                            yield
                            note(sub_strip())
                            yield
                            note(diag_strip())
                            yield
                            for i in range(1, 2 * c):
                                a_s, pieces = full_strip(i)
                                note((i, a_s, pieces))
                                yield
                        # flush: last pending carries the stop flag
                        while len(pending) > 1:
                            emit_out()
                        emit_out(last=True)
                        yield

                    for _ in causal_steps():
                        fill()
                else:
                    for i in range(ni):
                        a1, a2 = blk_scores(i, 0, SQ)
                        a_s = apool.tile([128, SQ], BF16, tag="a")
                        if variant == "generic":
                            gm = gmp.tile([128, SQ], F32, tag="gm")
                            nc.sync.dma_start(gm[:], gmask.ap()[c, i])
                            t = s1p.tile([128, SQ], F32, tag="s1d")
                            nc.vector.tensor_mul(t[:], a1[:], gm[:])
                            nc.vector.tensor_mul(a_s[:], t[:], a2[:])
                        else:
                            t = s1p.tile([128, SQ], F32, tag="s1d")
                            nc.scalar.copy(t[:], a1[:])
                            nc.vector.tensor_mul(a_s[:], t[:], a2[:])
                        nc.tensor.matmul(
                            o_ps[:], va[:, TB * i : TB * (i + 1)], a_s[:],
                            start=(i == 0), stop=(i == ni - 1),
                        )
                        fill()
                if bw is not None:
                    for _ in bw:
                        pass
                # drain any remaining epilogue steps of the previous chunk
                if prev_epi is not None:
                    for _ in prev_epi:
                        pass
                prev_epi = epilogue(c, o_ps)
                # kick off the first steps (o_sb copy) right away
                next(prev_epi, None)

            for _ in prev_epi:
                pass

    nc.compile()
    return nc


def _perm_blocks(c, p):
    """Order of the 4 query blocks of chunk c: parity-p blocks first,
    then the other parity in descending order (mask-shape alignment)."""
    return [4 * c + p, 4 * c + 2 + p, 4 * c + 3 - p, 4 * c + 1 - p]


def _host_inputs(x, cos, sin, Wq1, Wq2, Wk1, Wk2, Wv, Wo, variant, mask):
    wimg = np.empty((128, WIMG_COLS), np.float32)
    for name, w in (("q1", Wq1), ("q2", Wq2), ("k1", Wk1), ("k2", Wk2),
                    ("v", Wv * (1.0 / DH))):
        off = WOFF[name]
        # wimg[p_, off + j*128 + dcol] = w[j*128 + p_, dcol]
        wimg[:, off : off + D] = (
            w.reshape(DC, 128, DH).transpose(1, 0, 2).reshape(128, D)
        )
    wimg[:, WO_OFF:] = Wo  # [128 d, D]
    wimg = wimg.astype(ml_dtypes.bfloat16)
    ident = np.eye(128, dtype=np.float32)

    # tri-keep: key g*128+t visible to query g*128+cc iff t <= cc
    tt = np.arange(128)[:, None]
    ccol = np.arange(128)[None, :]
    tri = (tt <= ccol).astype(np.float32)

    in_maps = []
    perms = []
    for core in range(8):
        b, p = divmod(core, 2)
        blocks = np.concatenate(
            [np.asarray(_perm_blocks(c, p)) for c in range(NCH)]
        )
        qperm = (blocks[:, None] * 128 + np.arange(128)[None, :]).reshape(-1)
        perms.append(qperm)
        xsel = x[b][qperm]  # [S, D]
        xq = np.ascontiguousarray(
            xsel.reshape(NCH, SQ, DC, 128).transpose(0, 3, 2, 1)
        ).reshape(NCH, 128, DC * SQ).astype(ml_dtypes.bfloat16)
        csarr = np.empty((NCH, 128, 2 * SQ), np.float32)
        cosl = cos[qperm].reshape(NCH, SQ, 64).transpose(0, 2, 1)
        sinl = sin[qperm].reshape(NCH, SQ, 64).transpose(0, 2, 1)
        csarr[:, 0:64, 0:SQ] = cosl
        csarr[:, 64:128, 0:SQ] = cosl
        # rope via partition swap: rot = [x2; x1], so the sin multiplier
        # is [-sin; sin] (negation folded into the table)
        csarr[:, 0:64, SQ:] = -sinl
        csarr[:, 64:128, SQ:] = sinl
        csarr = csarr.astype(ml_dtypes.bfloat16)
        m = {"xq": xq, "cs": csarr, "wimg": wimg, "iden": ident}
        if variant == "causal":
            X = np.full((128, 128), 1.0 - p, np.float32)
            m["dmask"] = np.concatenate([tri, X], axis=1)
            m["dmaskw"] = np.concatenate(
                [np.zeros((128, 128), np.float32), tri, X,
                 np.zeros((128, 128), np.float32)], axis=1)
        elif variant == "generic":
            # mult[s, t] = 0 where mask True; per core: [NCH, NSTRIP, 128, SQ]
            mult = (~mask).astype(np.float32)  # [S(q), S(k)]
            gm = np.empty((NCH, NSTRIP, 128, SQ), np.float32)
            for c in range(NCH):
                qcols = qperm[c * SQ : (c + 1) * SQ]
                sub = mult[qcols][:, :].T  # [S(k), SQ]
                for i in range(NSTRIP):
                    kb = 2 * i + p
                    gm[c, i] = sub[kb * 128 : (kb + 1) * 128, :]
            m["gmask"] = gm
        in_maps.append(m)
    return in_maps, perms


def kernel(x, cos, sin, causal_mask, Wq1, Wq2, Wk1, Wk2, Wv, Wo):
    x = np.ascontiguousarray(np.asarray(x, dtype=np.float32))
    cos = np.asarray(cos, dtype=np.float32)
    sin = np.asarray(sin, dtype=np.float32)
    mask = np.asarray(causal_mask, dtype=bool)
    args = [np.asarray(w, dtype=np.float32)
            for w in (Wq1, Wq2, Wk1, Wk2, Wv, Wo)]

    if not mask.any():
        variant = "dense"
    else:
        triu = np.triu(np.ones((S, S), dtype=bool), k=1)
        variant = "causal" if np.array_equal(mask, triu) else "generic"

    if variant not in _compiled:
        _compiled[variant] = _build(variant)
    nc = _compiled[variant]

    in_maps, perms = _host_inputs(x, cos, sin, *args, variant, mask)
    res = run_bass_kernel_spmd(nc, in_maps, list(range(8)))

    out = np.empty((B, S, D), np.float32)
    for b in range(B):
        acc = None
        for p in range(2):
            core = 2 * b + p
            yc = (np.asarray(res.results[core]["y"], dtype=np.float32)
                  .reshape(NCH, 128, SQ // 128, D)
                  .transpose(0, 2, 1, 3)
                  .reshape(S, D))
            inv = np.empty(S, np.int64)
            inv[perms[core]] = np.arange(S)
            contrib = yc[inv]
            acc = contrib if acc is None else acc + contrib
        out[b] = acc
    return out


# revision 32
# speedup vs baseline: 1.0479x; 1.0479x over previous
"""Bilinear attention (a = causal(a1*a2), no softmax) on 8 Trainium2 cores.

Sharding: core = (batch, key-parity). Each core processes all queries of its
batch against the even- or odd-indexed 128-key blocks. With causal masking,
query-chunk c (512 queries) needs exactly 2c+2 parity-strip key blocks on
either parity, so a single SPMD program covers both cores of a pair; the
per-core difference lives entirely in host-side data layout. Partial outputs
(each pair member saw half the keys) are summed on host.

v4 (bf16 + pipelining): all matmul inputs are bfloat16 (PE at 1.0 cyc/row
for any free dim, input DMA halved, SBUF traffic halved). PSUM stays fp32;
the fp32->bf16 casts ride the PSUM->SBUF copies. The RoPE rotation is a
partition-half swap done with two SBUF->SBUF DMAs (negation folded into a
host-side [-sin; sin] table), freeing the PE of all rotation matmuls; the
rope multiplies/adds are pure-bf16 on DVE/GpSimd. The strip loop is
software-pipelined with a 2-strip lookahead (5 score PSUM banks + 1 out
bank): out(i) is issued on PE after the score matmuls of strip i+2, hiding
the Act-copy + DVE-mul combine chain (~1.5us) behind two strips of PE
work. o_ps uses a single PSUM accumulation group (start on the first
piece, bank-wide pending-zero consumed lazily). The Wo/y epilogue of chunk
c is a generator interleaved into chunk c+1's strip loop after the
boundary generator is exhausted (fills the last chunk's strip stalls,
shortens the tail).

Device layout is transpose-free: the host ships x^T per query chunk with the
4 query blocks permuted [4c+p, 4c+2+p, 4c+3-p, 4c+1-p] (parity-first, then
descending), so the kv projections read the first 256 columns of the same
tile the q projections read. Projections produce q^T/k^T [d, s] tiles;
scores are computed transposed (a^T[t, sq] = matmul(lhsT=k^T block,
rhs=q^T)); v comes out of a PE transpose as [t, d]; out^T[d, sq]
accumulates in PSUM over key blocks and feeds the Wo matmul as stationary.
"""

import sys

if "/opt/trn_rl_repo" not in sys.path:
    sys.path.insert(0, "/opt/trn_rl_repo")

import numpy as np
import ml_dtypes

import concourse.bass as bass
import concourse.mybir as mybir
import concourse.tile as tile
from concourse import bacc
from concourse.bass_utils import run_bass_kernel_spmd

B, S, D, DH = 4, 4096, 1024, 128
NCH = 8          # query chunks per batch
SQ = S // NCH    # 512 queries per chunk
TB = 128         # key block
NSTRIP = S // TB // 2  # 16 strip blocks per core
DC = D // 128    # 8 contraction chunks
F32 = mybir.dt.float32
F32R = mybir.dt.float32r
BF16 = mybir.dt.bfloat16

# wimg column offsets: k1 k2 v q1 q2 (each DC*128 cols) then Wo (D cols)
WOFF = {name: i * D for i, name in enumerate(["k1", "k2", "v", "q1", "q2"])}
WO_OFF = 5 * D
WIMG_COLS = 5 * D + D

_compiled = {}


def _edge(ap):
    """Strided view of a [128, 512] AP selecting cols {0:128} u {384:512}
    as a [128, 2, 128] access pattern."""
    a = ap[:, 0:128]
    return bass.AP(tensor=a.tensor, offset=a.offset,
                   ap=[a.ap[0], [384, 2], [1, 128]])


def _swap(nc, rot, src, n):
    """rot = partition-half swap of src ([x1;x2] -> [x2;x1]) via two
    SBUF->SBUF DMAs (issue cost split across two queues). The rope negation
    is folded into the host-side sin table ([-sin; sin])."""
    nc.gpsimd.dma_start(rot[0:64, :n], src[64:128, :n])
    nc.sync.dma_start(rot[64:128, :n], src[0:64, :n])


def _rope(nc, tmp, rsp, dst, src_sbuf, cs2, n):
    """dst(BF16) = rope(src) in [d, s] layout.

    rot = partition-swapped src (DMA), then
    dst = src*[cos;cos] + rot*[-sin;sin]. cs2: [128, 2*SQ] BF16 SBUF with
    [cos;cos] in cols 0:SQ and [-sin;sin] in cols SQ:2*SQ. src_sbuf BF16.
    Pure-bf16 muls on DVE, pure-bf16 add on GpSimd.
    """
    rot = rsp.tile([128, SQ], BF16, tag="rot")
    _swap(nc, rot, src_sbuf, n)
    m = tmp.tile([128, n], BF16, tag="ropetmp")
    nc.vector.tensor_mul(m[:], src_sbuf[:, :n], cs2[:, 0:n])
    t = tmp.tile([128, n], BF16, tag="ropetmp")
    nc.vector.tensor_mul(t[:], rot[:, :n], cs2[:, SQ : SQ + n])
    nc.vector.tensor_add(dst[:, :n], m[:], t[:])


def _rope2(nc, tmp, rsp, dst, src_sbuf, cs2):
    """Rope a [128, 512] (k1|k2) pair into BF16; both halves use kv-col
    cos/sin."""
    rot = rsp.tile([128, SQ], BF16, tag="rot")
    _swap(nc, rot, src_sbuf, SQ)
    # cs broadcast: repeat cs2[:, 0:256] (cc) / cs2[:, SQ:SQ+256] (ssn) twice
    def rep2(base_off):
        a = cs2[:, base_off : base_off + 256]
        return bass.AP(tensor=a.tensor, offset=a.offset,
                       ap=[a.ap[0], [0, 2], [1, 256]])
    m = tmp.tile([128, 512], BF16, tag="ropetmp")
    nc.vector.tensor_mul(m[:].rearrange("p (r n) -> p r n", r=2), src_sbuf[:].rearrange("p (r n) -> p r n", r=2), rep2(0))
    t = tmp.tile([128, 512], BF16, tag="ropetmp")
    nc.vector.tensor_mul(t[:].rearrange("p (r n) -> p r n", r=2), rot[:].rearrange("p (r n) -> p r n", r=2), rep2(SQ))
    nc.vector.tensor_add(dst[:], m[:], t[:])


def _proj(nc, psum, wimg, name, xq_t, ncols, start_col=0, psum_col=0):
    """Accumulate a [128, ncols] projection into psum over the DC chunks."""
    off = WOFF[name]
    for j in range(DC):
        nc.tensor.matmul(
            psum[:, psum_col : psum_col + ncols],
            wimg[:, off + j * 128 : off + (j + 1) * 128],
            xq_t[:, j, start_col : start_col + ncols],
            start=(j == 0),
            stop=(j == DC - 1),
        )


def _build(variant):
    """variant: 'causal' | 'dense' | 'generic'."""
    assert variant in ("causal", "dense", "generic")
    nc = bacc.Bacc("TRN2", target_bir_lowering=False, debug=False, num_devices=8)

    xq = nc.dram_tensor("xq", [NCH, 128, DC * SQ], BF16, kind="ExternalInput")
    cs = nc.dram_tensor("cs", [NCH, 128, 2 * SQ], BF16, kind="ExternalInput")
    wimg = nc.dram_tensor("wimg", [128, WIMG_COLS], BF16, kind="ExternalInput")
    iden = nc.dram_tensor("iden", [128, 128], F32R, kind="ExternalInput")
    if variant == "causal":
        # dmask: [128, 256] = [tri-keep | X] (X = ones on even cores, zeros
        # on odd); serves both the strided diag edge {0:128}u{384:512} and
        # the half-width sub strip cols 128:384.
        dmask = nc.dram_tensor("dmask", [128, 256], F32, kind="ExternalInput")
        # wide mask for chunk 0's sub strip (runs full width to own the
        # PSUM accumulation start): [0 | tri | X | 0]
        dmaskw = nc.dram_tensor("dmaskw", [128, SQ], F32, kind="ExternalInput")
    if variant == "generic":
        gmask = nc.dram_tensor(
            "gmask", [NCH, NSTRIP, 128, SQ], F32, kind="ExternalInput"
        )
    y = nc.dram_tensor("y", [NCH, 128, (SQ // 128) * D], BF16, kind="ExternalOutput")

    n_iter = (lambda c: 2 * c + 2) if variant == "causal" else (lambda c: NSTRIP)

    with tile.TileContext(nc) as tc:
        with (
            tc.tile_pool(name="consts", bufs=1) as consts,
            tc.tile_pool(name="kv", bufs=1) as kvpool,
            tc.tile_pool(name="xqp", bufs=3) as xqp,
            tc.tile_pool(name="csp", bufs=3) as csp,
            tc.tile_pool(name="qp", bufs=2) as qp,
            tc.tile_pool(name="ropetmp", bufs=6) as pool64,
            tc.tile_pool(name="ropesrc", bufs=4) as ropesrc,
            tc.tile_pool(name="rsp", bufs=3) as rsp,
            tc.tile_pool(name="ap", bufs=6) as apool,
            tc.tile_pool(name="s1p", bufs=6) as s1p,
            tc.tile_pool(name="osb", bufs=3) as osb,
            tc.tile_pool(name="ysb", bufs=3) as ysb,
            tc.tile_pool(name="gmp", bufs=3) as gmp,
            tc.tile_pool(name="sps", bufs=5, space="PSUM") as sps,
            tc.tile_pool(name="bps", bufs=2, space="PSUM") as bps,
            tc.tile_pool(name="ops", bufs=1, space="PSUM") as ops,
        ):
            wimg_t = consts.tile([128, WIMG_COLS], BF16)
            iden_t = consts.tile([128, 128], F32R)
            if variant == "causal":
                dmask_t = consts.tile([128, 256], F32)
                dmaskw_t = consts.tile([128, SQ], F32)

            def load_chunk(c):
                # kv halves (cols 0:256 of each contraction piece) first:
                # the kv projections of the boundary run a chunk before the
                # q projections need the rest. Issues alternate between the
                # sync and gpsimd queues to halve sequencer serialization.
                xq_t = xqp.tile([128, DC, SQ], BF16, tag="xq")
                src = xq.ap()[c].rearrange("p (j n) -> p j n", j=DC)
                nc.sync.dma_start(xq_t[:, 0:4, 0:256], src[:, 0:4, 0:256])
                nc.sync.dma_start(xq_t[:, 4:8, 0:256], src[:, 4:8, 0:256])
                cs_t = csp.tile([128, 2 * SQ], BF16, tag="cs")
                nc.sync.dma_start(cs_t[:], cs.ap()[c])
                nc.sync.dma_start(xq_t[:, 0:4, 256:512], src[:, 0:4, 256:512])
                nc.sync.dma_start(xq_t[:, 4:8, 256:512], src[:, 4:8, 256:512])
                return xq_t, cs_t

            def load_consts_ordered(first_io):
                # prologue: spread issues over four queues so the DMA-issue
                # train (~0.7us per dma_start on one sequencer) doesn't gate
                # the first projections; first pieces are the j<4 half of
                # the k1 weights and the first chunk-0 kv piece
                nc.sync.dma_start(wimg_t[:, : D // 2], wimg.ap()[:, : D // 2])
                io0 = None
                if first_io is not None:
                    xq_t = xqp.tile([128, DC, SQ], BF16, tag="xq")
                    src = xq.ap()[0].rearrange("p (j n) -> p j n", j=DC)
                    nc.gpsimd.dma_start(xq_t[:, 0:4, 0:256], src[:, 0:4, 0:256])
                nc.scalar.dma_start(wimg_t[:, D // 2 : D], wimg.ap()[:, D // 2 : D])
                if first_io is not None:
                    nc.scalar.dma_start(xq_t[:, 4:8, 0:256], src[:, 4:8, 0:256])
                    cs_t = csp.tile([128, 2 * SQ], BF16, tag="cs")
                    nc.gpsimd.dma_start(cs_t[:], cs.ap()[0])
                nc.sync.dma_start(wimg_t[:, D : 3 * D], wimg.ap()[:, D : 3 * D])
                nc.gpsimd.dma_start(wimg_t[:, 3 * D : 5 * D],
                                    wimg.ap()[:, 3 * D : 5 * D])
                if first_io is not None:
                    nc.scalar.dma_start(xq_t[:, 0:4, 256:512],
                                        src[:, 0:4, 256:512])
                    nc.gpsimd.dma_start(xq_t[:, 4:8, 256:512],
                                        src[:, 4:8, 256:512])
                    io0 = (xq_t, cs_t)
                nc.sync.dma_start(iden_t[:], iden.ap())
                if variant == "causal":
                    nc.gpsimd.dma_start(dmask_t[:], dmask.ap())
                    nc.scalar.dma_start(dmaskw_t[:], dmaskw.ap())
                nc.sync.dma_start(wimg_t[:, 5 * D :], wimg.ap()[:, 5 * D :])
                return io0

            k12a = kvpool.tile([128, NCH * 512], BF16, tag="k12a")
            va = kvpool.tile([128, 2 * NSTRIP * TB], BF16, tag="va")

            def k1_sl(i):
                return k12a[:, (i // 2) * 512 + (i % 2) * TB :][:, :TB]

            def k2_sl(i):
                return k12a[:, (i // 2) * 512 + 256 + (i % 2) * TB :][:, :TB]

            def boundary(c):
                """Generator: emits chunk-c kv/q projections in steps.
                Order: k, q1, q2, then v — the q ropes' swap-DMA latency
                hides under the v stage before chunk c's strips start."""
                xq_t, cs_t = chunk_io.pop(c)
                if variant == "causal":
                    kps = bps.tile([128, 512], F32, tag="bps")
                    _proj(nc, kps, wimg_t, "k1", xq_t, 256)
                    yield
                    _proj(nc, kps, wimg_t, "k2", xq_t, 256, psum_col=256)
                    ksb = ropesrc.tile([128, 512], BF16, tag="ropesrc")
                    nc.scalar.copy(ksb[:], kps[:])
                    yield
                    _rope2(nc, pool64, rsp, k12a[:, 512 * c : 512 * (c + 1)],
                           ksb, cs_t)
                    yield
                qs = []
                for name in ("q1", "q2"):
                    qps = bps.tile([128, 512], F32, tag="bps")
                    _proj(nc, qps, wimg_t, name, xq_t, SQ)
                    qsb = ropesrc.tile([128, SQ], BF16, tag="ropesrc")
                    nc.scalar.copy(qsb[:], qps[:])
                    yield
                    qdst = qp.tile([128, SQ], BF16, tag=f"{name}s")
                    _rope(nc, pool64, rsp, qdst, qsb, cs_t, SQ)
                    qs.append(qdst)
                    yield
                chunk_q[c] = qs
                if variant == "causal":
                    vps = bps.tile([128, 512], F32, tag="bps")
                    _proj(nc, vps, wimg_t, "v", xq_t, 256)
                    vsb = ropesrc.tile([128, 256], F32R, tag="vsb")
                    nc.scalar.copy(vsb[:], vps[:, :256])
                    yield
                    vtp = bps.tile([128, 512], F32R, tag="bps")
                    nc.tensor.transpose(vtp[:, 0:128], vsb[:, 0:128], iden_t[:])
                    nc.tensor.transpose(vtp[:, 128:256], vsb[:, 128:256], iden_t[:])
                    nc.scalar.copy(va[:, 256 * c : 256 * (c + 1)], vtp[:, 0:256])
                    yield

            def epilogue(c, o_ps):
                """Generator: Wo projection + y store for chunk c."""
                o_sb = osb.tile([128, SQ], BF16, tag="osb")
                nc.scalar.copy(o_sb[:], o_ps[:])
                yield
                y_sb = ysb.tile([128, (SQ // 128) * D], BF16, tag="ysb")
                for r in range(SQ // 128):
                    for h in range(D // 512):
                        y_ps = sps.tile([128, 512], F32, tag="sps")
                        nc.tensor.matmul(
                            y_ps[:],
                            o_sb[:, 128 * r : 128 * (r + 1)],
                            wimg_t[:, WO_OFF + 512 * h : WO_OFF + 512 * (h + 1)],
                            start=True, stop=True,
                        )
                        if (r + h) % 2 == 0:
                            nc.vector.tensor_copy(
                                y_sb[:, r * D + 512 * h : r * D + 512 * (h + 1)],
                                y_ps[:])
                        else:
                            nc.scalar.copy(
                                y_sb[:, r * D + 512 * h : r * D + 512 * (h + 1)],
                                y_ps[:])
                        yield
                    # alternate output queues so the final drain is split;
                    # the last chunk uses the otherwise-idle scalar queue so
                    # its stores aren't FIFO'd behind earlier chunks' y
                    if c == NCH - 1:
                        eng = nc.scalar
                    else:
                        eng = nc.gpsimd if r % 2 == 0 else nc.sync
                    eng.dma_start(
                        y.ap()[c][:, r * D : (r + 1) * D],
                        y_sb[:, r * D : (r + 1) * D])

            chunk_q = {}
            chunk_io = {}

            if variant != "causal":
                # project all kv first (dense needs late key blocks early)
                load_consts_ordered(None)
                for c in range(NCH):
                    xq_t, cs_t = load_chunk(c)
                    kps = bps.tile([128, 512], F32, tag="bps")
                    _proj(nc, kps, wimg_t, "k1", xq_t, 256)
                    _proj(nc, kps, wimg_t, "k2", xq_t, 256, psum_col=256)
                    ksb = ropesrc.tile([128, 512], BF16, tag="ropesrc")
                    nc.scalar.copy(ksb[:], kps[:])
                    _rope2(nc, pool64, rsp, k12a[:, 512 * c : 512 * (c + 1)],
                           ksb, cs_t)
                    vps = bps.tile([128, 512], F32, tag="bps")
                    _proj(nc, vps, wimg_t, "v", xq_t, 256)
                    vsb = ropesrc.tile([128, 256], F32R, tag="vsb")
                    nc.scalar.copy(vsb[:], vps[:, :256])
                    vtp = bps.tile([128, 512], F32R, tag="bps")
                    nc.tensor.transpose(vtp[:, 0:128], vsb[:, 0:128], iden_t[:])
                    nc.tensor.transpose(vtp[:, 128:256], vsb[:, 128:256], iden_t[:])
                    nc.scalar.copy(va[:, 256 * c : 256 * (c + 1)], vtp[:, 0:256])

            # prologue: load chunk-0 inputs + weights in service order,
            # then chunk 1, then run chunk-0 boundary
            chunk_io[0] = load_consts_ordered(True)
            chunk_io[1] = load_chunk(1)
            for _ in boundary(0):
                pass

            prev_epi = None  # epilogue generator of the previous chunk
            for c in range(NCH):
                if c + 2 < NCH:
                    chunk_io[c + 2] = load_chunk(c + 2)
                bw = boundary(c + 1) if c + 1 < NCH else None
                q1s, q2s = chunk_q.pop(c)
                o_ps = ops.tile([128, SQ], F32, tag="ops")
                ni = n_iter(c)

                def blk_scores(i, qcol0, qcol1):
                    a1 = sps.tile([128, SQ], F32, tag="sps")
                    nc.tensor.matmul(a1[:, qcol0:qcol1], k1_sl(i),
                                     q1s[:, qcol0:qcol1], start=True, stop=True)
                    a2 = sps.tile([128, SQ], F32, tag="sps")
                    nc.tensor.matmul(a2[:, qcol0:qcol1], k2_sl(i),
                                     q2s[:, qcol0:qcol1], start=True, stop=True)
                    return a1, a2

                def fill():
                    """Advance background work: up to two boundary steps of
                    the next chunk (so its rope chains finish well before the
                    chunk transition), else one previous-epilogue step."""
                    if bw is not None:
                        if next(bw, "done") != "done":
                            next(bw, None)
                            return
                    if prev_epi is not None:
                        next(prev_epi, None)

                if variant == "causal":
                    # One PSUM accumulation group per o_ps bank: start=True
                    # only on the first piece (bank-wide pending-zero is
                    # consumed lazily by the other pieces' first writes),
                    # stop=True only on the very last piece.
                    started = [False]
                    # pending out halves: list of (strip, a_s, [(q0,q1),...])
                    pending = []

                    def emit_out(last=False):
                        if not pending:
                            return
                        i, a_s, pieces = pending.pop(0)
                        for pi, (q0, q1v) in enumerate(pieces):
                            st = not started[0]
                            started[0] = True
                            nc.tensor.matmul(
                                o_ps[:, q0:q1v], va[:, TB * i : TB * (i + 1)],
                                a_s[:, q0:q1v], start=st,
                                stop=(last and pi == len(pieces) - 1),
                            )

                    def note(entry):
                        # 2-strip lookahead: out(i) is issued after the score
                        # matmuls of strip i+2 (5 sps banks hold 2.5 strips)
                        if len(pending) >= 2:
                            emit_out()
                        pending.append(entry)

                    def full_strip(i):
                        """Scores + combine for an unmasked strip."""
                        a1, a2 = blk_scores(i, 0, SQ)
                        a_s = apool.tile([128, SQ], BF16, tag="a")
                        t = s1p.tile([128, SQ], F32, tag="s1")
                        nc.scalar.copy(t[:], a1[:])
                        nc.vector.tensor_mul(a_s[:], t[:], a2[:])
                        return a_s, [(0, 512)]

                    def sub_strip():
                        """The half-masked strip (keys 2c+1): only query cols
                        128:384 are live."""
                        i = 2 * c + 1
                        a1, a2 = blk_scores(i, 128, 384)
                        a_s = apool.tile([128, SQ], BF16, tag="a")
                        t = s1p.tile([128, 256], F32, tag="s1h")
                        nc.vector.tensor_mul(t[:], a1[:, 128:384], dmask_t[:])
                        nc.vector.tensor_mul(a_s[:, 128:384], t[:],
                                             a2[:, 128:384])
                        return i, a_s, [(128, 384)]

                    def diag_strip():
                        """Diag strip: mask only the edge cols
                        {0:128} u {384:512}."""
                        i = 2 * c
                        a1, a2 = blk_scores(i, 0, SQ)
                        t = s1p.tile([128, SQ], F32, tag="s1d")
                        dmv = dmask_t[:].rearrange("p (r n) -> p r n", r=2)
                        nc.vector.tensor_mul(
                            t[:, 0:256].rearrange("p (r n) -> p r n", r=2),
                            _edge(a1), dmv)
                        a_s = apool.tile([128, SQ], BF16, tag="a")
                        nc.vector.tensor_mul(
                            _edge(a_s),
                            t[:, 0:256].rearrange("p (r n) -> p r n", r=2),
                            _edge(a2))
                        nc.scalar.copy(t[:, 256:512], a1[:, 128:384])
                        nc.vector.tensor_mul(a_s[:, 128:384], t[:, 256:512],
                                             a2[:, 128:384])
                        return i, a_s, [(0, 512)]

                    def causal_steps():
                        if c == 0:
                            # chunk 0's sub strip runs full width with the
                            # wide mask to own the PSUM start
                            a1, a2 = blk_scores(1, 0, SQ)
                            a_s = apool.tile([128, SQ], BF16, tag="a")
                            t = s1p.tile([128, SQ], F32, tag="s1")
                            nc.vector.tensor_mul(t[:], a1[:], dmaskw_t[:])
                            nc.vector.tensor_mul(a_s[:], t[:], a2[:])
                            note((1, a_s, [(0, 512)]))
                            yield
                            note(diag_strip())
                            yield
                        else:
                            # order: full0 (owns PSUM start), sub, diag,
                            # remaining fulls (last full has the shortest
                            # combine chain -> minimal tail before epilogue)
                            a_s, pieces = full_strip(0)
                            note((0, a_s, pieces))
                            yield
                            note(sub_strip())
                            yield
                            note(diag_strip())
                            yield
                            for i in range(1, 2 * c):
                                a_s, pieces = full_strip(i)
                                note((i, a_s, pieces))
                                yield
                        # flush: last pending carries the stop flag
                        while len(pending) > 1:
                            emit_out()
                        emit_out(last=True)
                        yield

                    for _ in causal_steps():
                        fill()
                else:
                    for i in range(ni):
                        a1, a2 = blk_scores(i, 0, SQ)
                        a_s = apool.tile([128, SQ], BF16, tag="a")
                        if variant == "generic":
                            gm = gmp.tile([128, SQ], F32, tag="gm")
                            nc.sync.dma_start(gm[:], gmask.ap()[c, i])
                            t = s1p.tile([128, SQ], F32, tag="s1d")
                            nc.vector.tensor_mul(t[:], a1[:], gm[:])
                            nc.vector.tensor_mul(a_s[:], t[:], a2[:])
                        else:
                            t = s1p.tile([128, SQ], F32, tag="s1d")
                            nc.scalar.copy(t[:], a1[:])
                            nc.vector.tensor_mul(a_s[:], t[:], a2[:])
                        nc.tensor.matmul(
                            o_ps[:], va[:, TB * i : TB * (i + 1)], a_s[:],
                            start=(i == 0), stop=(i == ni - 1),
                        )
                        fill()
                if bw is not None:
                    for _ in bw:
                        pass
                # drain any remaining epilogue steps of the previous chunk
                if prev_epi is not None:
                    for _ in prev_epi:
                        pass
                prev_epi = epilogue(c, o_ps)
                # kick off the first steps (o_sb copy) right away
                next(prev_epi, None)

            for _ in prev_epi:
                pass

    nc.compile()
    return nc


def _perm_blocks(c, p):
    """Order of the 4 query blocks of chunk c: parity-p blocks first,
    then the other parity in descending order (mask-shape alignment)."""
    return [4 * c + p, 4 * c + 2 + p, 4 * c + 3 - p, 4 * c + 1 - p]


def _host_inputs(x, cos, sin, Wq1, Wq2, Wk1, Wk2, Wv, Wo, variant, mask):
    wimg = np.empty((128, WIMG_COLS), np.float32)
    for name, w in (("q1", Wq1), ("q2", Wq2), ("k1", Wk1), ("k2", Wk2),
                    ("v", Wv * (1.0 / DH))):
        off = WOFF[name]
        # wimg[p_, off + j*128 + dcol] = w[j*128 + p_, dcol]
        wimg[:, off : off + D] = (
            w.reshape(DC, 128, DH).transpose(1, 0, 2).reshape(128, D)
        )
    wimg[:, WO_OFF:] = Wo  # [128 d, D]
    wimg = wimg.astype(ml_dtypes.bfloat16)
    ident = np.eye(128, dtype=np.float32)

    # tri-keep: key g*128+t visible to query g*128+cc iff t <= cc
    tt = np.arange(128)[:, None]
    ccol = np.arange(128)[None, :]
    tri = (tt <= ccol).astype(np.float32)

    in_maps = []
    perms = []
    for core in range(8):
        b, p = divmod(core, 2)
        blocks = np.concatenate(
            [np.asarray(_perm_blocks(c, p)) for c in range(NCH)]
        )
        qperm = (blocks[:, None] * 128 + np.arange(128)[None, :]).reshape(-1)
        perms.append(qperm)
        xsel = x[b][qperm]  # [S, D]
        xq = np.ascontiguousarray(
            xsel.reshape(NCH, SQ, DC, 128).transpose(0, 3, 2, 1)
        ).reshape(NCH, 128, DC * SQ).astype(ml_dtypes.bfloat16)
        csarr = np.empty((NCH, 128, 2 * SQ), np.float32)
        cosl = cos[qperm].reshape(NCH, SQ, 64).transpose(0, 2, 1)
        sinl = sin[qperm].reshape(NCH, SQ, 64).transpose(0, 2, 1)
        csarr[:, 0:64, 0:SQ] = cosl
        csarr[:, 64:128, 0:SQ] = cosl
        # rope via partition swap: rot = [x2; x1], so the sin multiplier
        # is [-sin; sin] (negation folded into the table)
        csarr[:, 0:64, SQ:] = -sinl
        csarr[:, 64:128, SQ:] = sinl
        csarr = csarr.astype(ml_dtypes.bfloat16)
        m = {"xq": xq, "cs": csarr, "wimg": wimg, "iden": ident}
        if variant == "causal":
            X = np.full((128, 128), 1.0 - p, np.float32)
            m["dmask"] = np.concatenate([tri, X], axis=1)
            m["dmaskw"] = np.concatenate(
                [np.zeros((128, 128), np.float32), tri, X,
                 np.zeros((128, 128), np.float32)], axis=1)
        elif variant == "generic":
            # mult[s, t] = 0 where mask True; per core: [NCH, NSTRIP, 128, SQ]
            mult = (~mask).astype(np.float32)  # [S(q), S(k)]
            gm = np.empty((NCH, NSTRIP, 128, SQ), np.float32)
            for c in range(NCH):
                qcols = qperm[c * SQ : (c + 1) * SQ]
                sub = mult[qcols][:, :].T  # [S(k), SQ]
                for i in range(NSTRIP):
                    kb = 2 * i + p
                    gm[c, i] = sub[kb * 128 : (kb + 1) * 128, :]
            m["gmask"] = gm
        in_maps.append(m)
    return in_maps, perms


def kernel(x, cos, sin, causal_mask, Wq1, Wq2, Wk1, Wk2, Wv, Wo):
    x = np.ascontiguousarray(np.asarray(x, dtype=np.float32))
    cos = np.asarray(cos, dtype=np.float32)
    sin = np.asarray(sin, dtype=np.float32)
    mask = np.asarray(causal_mask, dtype=bool)
    args = [np.asarray(w, dtype=np.float32)
            for w in (Wq1, Wq2, Wk1, Wk2, Wv, Wo)]

    if not mask.any():
        variant = "dense"
    else:
        triu = np.triu(np.ones((S, S), dtype=bool), k=1)
        variant = "causal" if np.array_equal(mask, triu) else "generic"

    if variant not in _compiled:
        _compiled[variant] = _build(variant)
    nc = _compiled[variant]

    in_maps, perms = _host_inputs(x, cos, sin, *args, variant, mask)
    res = run_bass_kernel_spmd(nc, in_maps, list(range(8)))

    out = np.empty((B, S, D), np.float32)
    for b in range(B):
        acc = None
        for p in range(2):
            core = 2 * b + p
            yc = (np.asarray(res.results[core]["y"], dtype=np.float32)
                  .reshape(NCH, 128, SQ // 128, D)
                  .transpose(0, 2, 1, 3)
                  .reshape(S, D))
            inv = np.empty(S, np.int64)
            inv[perms[core]] = np.arange(S)
            contrib = yc[inv]
            acc = contrib if acc is None else acc + contrib
        out[b] = acc
    return out


# revision 33
# speedup vs baseline: 1.1232x; 1.0719x over previous
"""Bilinear attention (a = causal(a1*a2), no softmax) on 8 Trainium2 cores.

Sharding: core = (batch, key-parity). Each core processes all queries of its
batch against the even- or odd-indexed 128-key blocks. With causal masking,
query-chunk c (512 queries) needs exactly 2c+2 parity-strip key blocks on
either parity, so a single SPMD program covers both cores of a pair; the
per-core difference lives entirely in host-side data layout. Partial outputs
(each pair member saw half the keys) are summed on host.

v4 (bf16 + pipelining): all matmul inputs are bfloat16 (PE at 1.0 cyc/row
for any free dim, input DMA halved, SBUF traffic halved). PSUM stays fp32;
the fp32->bf16 casts ride the PSUM->SBUF copies. The RoPE rotation is a
partition-half swap done with two SBUF->SBUF DMAs (negation folded into a
host-side [-sin; sin] table), freeing the PE of all rotation matmuls; the
rope multiplies/adds are pure-bf16 on DVE/GpSimd. The strip loop is
software-pipelined with a 2-strip lookahead (5 score PSUM banks + 1 out
bank): out(i) is issued on PE after the score matmuls of strip i+2, hiding
the Act-copy + DVE-mul combine chain (~1.5us) behind two strips of PE
work. o_ps uses a single PSUM accumulation group (start on the first
piece, bank-wide pending-zero consumed lazily). The Wo/y epilogue of chunk
c is a generator interleaved into chunk c+1's strip loop after the
boundary generator is exhausted (fills the last chunk's strip stalls,
shortens the tail).

Device layout is transpose-free: the host ships x^T per query chunk with the
4 query blocks permuted [4c+p, 4c+2+p, 4c+3-p, 4c+1-p] (parity-first, then
descending), so the kv projections read the first 256 columns of the same
tile the q projections read. Projections produce q^T/k^T [d, s] tiles;
scores are computed transposed (a^T[t, sq] = matmul(lhsT=k^T block,
rhs=q^T)); v comes out of a PE transpose as [t, d]; out^T[d, sq]
accumulates in PSUM over key blocks and feeds the Wo matmul as stationary.
"""

import sys

if "/opt/trn_rl_repo" not in sys.path:
    sys.path.insert(0, "/opt/trn_rl_repo")

import numpy as np
import ml_dtypes

import concourse.bass as bass
import concourse.mybir as mybir
import concourse.tile as tile
from concourse import bacc
from concourse.bass_utils import run_bass_kernel_spmd

B, S, D, DH = 4, 4096, 1024, 128
NCH = 8          # query chunks per batch
SQ = S // NCH    # 512 queries per chunk
TB = 128         # key block
NSTRIP = S // TB // 2  # 16 strip blocks per core
DC = D // 128    # 8 contraction chunks
F32 = mybir.dt.float32
F32R = mybir.dt.float32r
BF16 = mybir.dt.bfloat16

# wimg column offsets: k1 k2 v q1 q2 (each DC*128 cols) then Wo (D cols)
WOFF = {name: i * D for i, name in enumerate(["k1", "k2", "v", "q1", "q2"])}
WO_OFF = 5 * D
WIMG_COLS = 5 * D + D

_compiled = {}


def _edge(ap):
    """Strided view of a [128, 512] AP selecting cols {0:128} u {384:512}
    as a [128, 2, 128] access pattern."""
    a = ap[:, 0:128]
    return bass.AP(tensor=a.tensor, offset=a.offset,
                   ap=[a.ap[0], [384, 2], [1, 128]])


def _swap(nc, rot, src, n):
    """rot = partition-half swap of src ([x1;x2] -> [x2;x1]) via two
    SBUF->SBUF DMAs (issue cost split across two queues). The rope negation
    is folded into the host-side sin table ([-sin; sin])."""
    nc.gpsimd.dma_start(rot[0:64, :n], src[64:128, :n])
    nc.sync.dma_start(rot[64:128, :n], src[0:64, :n])


def _rope(nc, tmp, rsp, dst, src_sbuf, cs2, n):
    """dst(BF16) = rope(src) in [d, s] layout.

    rot = partition-swapped src (DMA), then
    dst = src*[cos;cos] + rot*[-sin;sin]. cs2: [128, 2*SQ] BF16 SBUF with
    [cos;cos] in cols 0:SQ and [-sin;sin] in cols SQ:2*SQ. src_sbuf BF16.
    Pure-bf16 muls on DVE, pure-bf16 add on GpSimd.
    """
    rot = rsp.tile([128, SQ], BF16, tag="rot")
    _swap(nc, rot, src_sbuf, n)
    m = tmp.tile([128, n], BF16, tag="ropetmp")
    nc.vector.tensor_mul(m[:], src_sbuf[:, :n], cs2[:, 0:n])
    t = tmp.tile([128, n], BF16, tag="ropetmp")
    nc.vector.tensor_mul(t[:], rot[:, :n], cs2[:, SQ : SQ + n])
    nc.vector.tensor_add(dst[:, :n], m[:], t[:])


def _rope2(nc, tmp, rsp, dst, src_sbuf, cs2):
    """Rope a [128, 512] (k1|k2) pair into BF16; both halves use kv-col
    cos/sin."""
    rot = rsp.tile([128, SQ], BF16, tag="rot")
    _swap(nc, rot, src_sbuf, SQ)
    # cs broadcast: repeat cs2[:, 0:256] (cc) / cs2[:, SQ:SQ+256] (ssn) twice
    def rep2(base_off):
        a = cs2[:, base_off : base_off + 256]
        return bass.AP(tensor=a.tensor, offset=a.offset,
                       ap=[a.ap[0], [0, 2], [1, 256]])
    m = tmp.tile([128, 512], BF16, tag="ropetmp")
    nc.vector.tensor_mul(m[:].rearrange("p (r n) -> p r n", r=2), src_sbuf[:].rearrange("p (r n) -> p r n", r=2), rep2(0))
    t = tmp.tile([128, 512], BF16, tag="ropetmp")
    nc.vector.tensor_mul(t[:].rearrange("p (r n) -> p r n", r=2), rot[:].rearrange("p (r n) -> p r n", r=2), rep2(SQ))
    nc.vector.tensor_add(dst[:], m[:], t[:])


def _proj(nc, psum, wimg, name, xq_t, ncols, start_col=0, psum_col=0):
    """Accumulate a [128, ncols] projection into psum over the DC chunks."""
    off = WOFF[name]
    for j in range(DC):
        nc.tensor.matmul(
            psum[:, psum_col : psum_col + ncols],
            wimg[:, off + j * 128 : off + (j + 1) * 128],
            xq_t[:, j, start_col : start_col + ncols],
            start=(j == 0),
            stop=(j == DC - 1),
        )


def _build(variant):
    """variant: 'causal' | 'dense' | 'generic'."""
    assert variant in ("causal", "dense", "generic")
    nc = bacc.Bacc("TRN2", target_bir_lowering=False, debug=False, num_devices=8)

    xq = nc.dram_tensor("xq", [NCH, 128, DC * SQ], BF16, kind="ExternalInput")
    cs = nc.dram_tensor("cs", [NCH, 128, 2 * SQ], BF16, kind="ExternalInput")
    wimg = nc.dram_tensor("wimg", [128, WIMG_COLS], BF16, kind="ExternalInput")
    iden = nc.dram_tensor("iden", [128, 128], F32R, kind="ExternalInput")
    if variant == "causal":
        # dmask: [128, 256] = [tri-keep | X] (X = ones on even cores, zeros
        # on odd); serves both the strided diag edge {0:128}u{384:512} and
        # the half-width sub strip cols 128:384.
        dmask = nc.dram_tensor("dmask", [128, 256], F32, kind="ExternalInput")
        # wide mask for chunk 0's sub strip (runs full width to own the
        # PSUM accumulation start): [0 | tri | X | 0]
        dmaskw = nc.dram_tensor("dmaskw", [128, SQ], F32, kind="ExternalInput")
    if variant == "generic":
        gmask = nc.dram_tensor(
            "gmask", [NCH, NSTRIP, 128, SQ], F32, kind="ExternalInput"
        )
    y = nc.dram_tensor("y", [NCH, 128, (SQ // 128) * D], BF16, kind="ExternalOutput")

    n_iter = (lambda c: 2 * c + 2) if variant == "causal" else (lambda c: NSTRIP)

    with tile.TileContext(nc) as tc:
        with (
            tc.tile_pool(name="consts", bufs=1) as consts,
            tc.tile_pool(name="kv", bufs=1) as kvpool,
            tc.tile_pool(name="xqp", bufs=3) as xqp,
            tc.tile_pool(name="csp", bufs=3) as csp,
            tc.tile_pool(name="qp", bufs=2) as qp,
            tc.tile_pool(name="ropetmp", bufs=6) as pool64,
            tc.tile_pool(name="ropesrc", bufs=4) as ropesrc,
            tc.tile_pool(name="rsp", bufs=3) as rsp,
            tc.tile_pool(name="ap", bufs=6) as apool,
            tc.tile_pool(name="s1p", bufs=6) as s1p,
            tc.tile_pool(name="osb", bufs=3) as osb,
            tc.tile_pool(name="ysb", bufs=3) as ysb,
            tc.tile_pool(name="gmp", bufs=3) as gmp,
            tc.tile_pool(name="sps", bufs=5, space="PSUM") as sps,
            tc.tile_pool(name="bps", bufs=2, space="PSUM") as bps,
            tc.tile_pool(name="ops", bufs=1, space="PSUM") as ops,
        ):
            wimg_t = consts.tile([128, WIMG_COLS], BF16)
            iden_t = consts.tile([128, 128], F32R)
            if variant == "causal":
                dmask_t = consts.tile([128, 256], F32)
                dmaskw_t = consts.tile([128, SQ], F32)

            def load_chunk(c):
                # kv halves (cols 0:256 of each contraction piece) first:
                # the kv projections of the boundary run a chunk before the
                # q projections need the rest. Issues alternate between the
                # sync and gpsimd queues to halve sequencer serialization.
                xq_t = xqp.tile([128, DC, SQ], BF16, tag="xq")
                src = xq.ap()[c].rearrange("p (j n) -> p j n", j=DC)
                nc.sync.dma_start(xq_t[:, 0:4, 0:256], src[:, 0:4, 0:256])
                nc.sync.dma_start(xq_t[:, 4:8, 0:256], src[:, 4:8, 0:256])
                cs_t = csp.tile([128, 2 * SQ], BF16, tag="cs")
                nc.sync.dma_start(cs_t[:], cs.ap()[c])
                nc.sync.dma_start(xq_t[:, 0:4, 256:512], src[:, 0:4, 256:512])
                nc.sync.dma_start(xq_t[:, 4:8, 256:512], src[:, 4:8, 256:512])
                return xq_t, cs_t

            def load_consts_ordered(first_io):
                # issue order = DMA service order: the j<4 half of the k1
                # weights and the first chunk-0 kv piece land first so the
                # very first projection matmul has minimal DMA to wait for
                nc.sync.dma_start(wimg_t[:, : D // 2], wimg.ap()[:, : D // 2])
                io0 = None
                if first_io is not None:
                    xq_t = xqp.tile([128, DC, SQ], BF16, tag="xq")
                    src = xq.ap()[0].rearrange("p (j n) -> p j n", j=DC)
                    nc.sync.dma_start(xq_t[:, 0:4, 0:256], src[:, 0:4, 0:256])
                nc.sync.dma_start(wimg_t[:, D // 2 : D], wimg.ap()[:, D // 2 : D])
                if first_io is not None:
                    nc.sync.dma_start(xq_t[:, 4:8, 0:256], src[:, 4:8, 0:256])
                    cs_t = csp.tile([128, 2 * SQ], BF16, tag="cs")
                    nc.sync.dma_start(cs_t[:], cs.ap()[0])
                nc.sync.dma_start(wimg_t[:, D : 3 * D], wimg.ap()[:, D : 3 * D])
                nc.sync.dma_start(wimg_t[:, 3 * D : 5 * D], wimg.ap()[:, 3 * D : 5 * D])
                if first_io is not None:
                    for j0 in range(0, DC, 4):
                        nc.sync.dma_start(xq_t[:, j0 : j0 + 4, 256:512],
                                          src[:, j0 : j0 + 4, 256:512])
                    io0 = (xq_t, cs_t)
                nc.sync.dma_start(iden_t[:], iden.ap())
                if variant == "causal":
                    nc.sync.dma_start(dmask_t[:], dmask.ap())
                    nc.sync.dma_start(dmaskw_t[:], dmaskw.ap())
                nc.sync.dma_start(wimg_t[:, 5 * D :], wimg.ap()[:, 5 * D :])
                return io0

            k12a = kvpool.tile([128, NCH * 512], BF16, tag="k12a")
            va = kvpool.tile([128, 2 * NSTRIP * TB], BF16, tag="va")

            def k1_sl(i):
                return k12a[:, (i // 2) * 512 + (i % 2) * TB :][:, :TB]

            def k2_sl(i):
                return k12a[:, (i // 2) * 512 + 256 + (i % 2) * TB :][:, :TB]

            def boundary(c):
                """Generator: emits chunk-c kv/q projections in steps.
                Order: k, q1, q2, then v — the q ropes' swap-DMA latency
                hides under the v stage before chunk c's strips start."""
                xq_t, cs_t = chunk_io.pop(c)
                if variant == "causal":
                    kps = bps.tile([128, 512], F32, tag="bps")
                    _proj(nc, kps, wimg_t, "k1", xq_t, 256)
                    yield
                    _proj(nc, kps, wimg_t, "k2", xq_t, 256, psum_col=256)
                    ksb = ropesrc.tile([128, 512], BF16, tag="ropesrc")
                    nc.scalar.copy(ksb[:], kps[:])
                    yield
                    _rope2(nc, pool64, rsp, k12a[:, 512 * c : 512 * (c + 1)],
                           ksb, cs_t)
                    yield
                qs = []
                for name in ("q1", "q2"):
                    qps = bps.tile([128, 512], F32, tag="bps")
                    _proj(nc, qps, wimg_t, name, xq_t, SQ)
                    qsb = ropesrc.tile([128, SQ], BF16, tag="ropesrc")
                    nc.scalar.copy(qsb[:], qps[:])
                    yield
                    qdst = qp.tile([128, SQ], BF16, tag=f"{name}s")
                    _rope(nc, pool64, rsp, qdst, qsb, cs_t, SQ)
                    qs.append(qdst)
                    yield
                chunk_q[c] = qs
                if variant == "causal":
                    vps = bps.tile([128, 512], F32, tag="bps")
                    _proj(nc, vps, wimg_t, "v", xq_t, 256)
                    vsb = ropesrc.tile([128, 256], F32R, tag="vsb")
                    nc.scalar.copy(vsb[:], vps[:, :256])
                    yield
                    vtp = bps.tile([128, 512], F32R, tag="bps")
                    nc.tensor.transpose(vtp[:, 0:128], vsb[:, 0:128], iden_t[:])
                    nc.tensor.transpose(vtp[:, 128:256], vsb[:, 128:256], iden_t[:])
                    nc.scalar.copy(va[:, 256 * c : 256 * (c + 1)], vtp[:, 0:256])
                    yield

            def epilogue(c, o_ps):
                """Generator: Wo projection + y store for chunk c."""
                o_sb = osb.tile([128, SQ], BF16, tag="osb")
                nc.scalar.copy(o_sb[:], o_ps[:])
                yield
                y_sb = ysb.tile([128, (SQ // 128) * D], BF16, tag="ysb")
                for r in range(SQ // 128):
                    for h in range(D // 512):
                        y_ps = sps.tile([128, 512], F32, tag="sps")
                        nc.tensor.matmul(
                            y_ps[:],
                            o_sb[:, 128 * r : 128 * (r + 1)],
                            wimg_t[:, WO_OFF + 512 * h : WO_OFF + 512 * (h + 1)],
                            start=True, stop=True,
                        )
                        if (r + h) % 2 == 0:
                            nc.vector.tensor_copy(
                                y_sb[:, r * D + 512 * h : r * D + 512 * (h + 1)],
                                y_ps[:])
                        else:
                            nc.scalar.copy(
                                y_sb[:, r * D + 512 * h : r * D + 512 * (h + 1)],
                                y_ps[:])
                        yield
                    # alternate output queues so the final drain is split;
                    # the last chunk uses the otherwise-idle scalar queue so
                    # its stores aren't FIFO'd behind earlier chunks' y
                    if c == NCH - 1:
                        eng = nc.scalar
                    else:
                        eng = nc.gpsimd if r % 2 == 0 else nc.sync
                    eng.dma_start(
                        y.ap()[c][:, r * D : (r + 1) * D],
                        y_sb[:, r * D : (r + 1) * D])

            chunk_q = {}
            chunk_io = {}

            if variant != "causal":
                # project all kv first (dense needs late key blocks early)
                load_consts_ordered(None)
                for c in range(NCH):
                    xq_t, cs_t = load_chunk(c)
                    kps = bps.tile([128, 512], F32, tag="bps")
                    _proj(nc, kps, wimg_t, "k1", xq_t, 256)
                    _proj(nc, kps, wimg_t, "k2", xq_t, 256, psum_col=256)
                    ksb = ropesrc.tile([128, 512], BF16, tag="ropesrc")
                    nc.scalar.copy(ksb[:], kps[:])
                    _rope2(nc, pool64, rsp, k12a[:, 512 * c : 512 * (c + 1)],
                           ksb, cs_t)
                    vps = bps.tile([128, 512], F32, tag="bps")
                    _proj(nc, vps, wimg_t, "v", xq_t, 256)
                    vsb = ropesrc.tile([128, 256], F32R, tag="vsb")
                    nc.scalar.copy(vsb[:], vps[:, :256])
                    vtp = bps.tile([128, 512], F32R, tag="bps")
                    nc.tensor.transpose(vtp[:, 0:128], vsb[:, 0:128], iden_t[:])
                    nc.tensor.transpose(vtp[:, 128:256], vsb[:, 128:256], iden_t[:])
                    nc.scalar.copy(va[:, 256 * c : 256 * (c + 1)], vtp[:, 0:256])

            # prologue: load chunk-0 inputs + weights in service order,
            # then chunk 1, then run chunk-0 boundary
            chunk_io[0] = load_consts_ordered(True)
            chunk_io[1] = load_chunk(1)
            for _ in boundary(0):
                pass

            prev_epi = None  # epilogue generator of the previous chunk
            for c in range(NCH):
                if c + 2 < NCH:
                    chunk_io[c + 2] = load_chunk(c + 2)
                bw = boundary(c + 1) if c + 1 < NCH else None
                q1s, q2s = chunk_q.pop(c)
                o_ps = ops.tile([128, SQ], F32, tag="ops")
                ni = n_iter(c)

                def blk_scores(i, qcol0, qcol1):
                    a1 = sps.tile([128, SQ], F32, tag="sps")
                    nc.tensor.matmul(a1[:, qcol0:qcol1], k1_sl(i),
                                     q1s[:, qcol0:qcol1], start=True, stop=True)
                    a2 = sps.tile([128, SQ], F32, tag="sps")
                    nc.tensor.matmul(a2[:, qcol0:qcol1], k2_sl(i),
                                     q2s[:, qcol0:qcol1], start=True, stop=True)
                    return a1, a2

                def fill():
                    """Advance background work: up to two boundary steps of
                    the next chunk (so its rope chains finish well before the
                    chunk transition), else one previous-epilogue step."""
                    if bw is not None:
                        if next(bw, "done") != "done":
                            next(bw, None)
                            return
                    if prev_epi is not None:
                        next(prev_epi, None)

                if variant == "causal":
                    # One PSUM accumulation group per o_ps bank: start=True
                    # only on the first piece (bank-wide pending-zero is
                    # consumed lazily by the other pieces' first writes),
                    # stop=True only on the very last piece.
                    started = [False]
                    # pending out halves: list of (strip, a_s, [(q0,q1),...])
                    pending = []

                    def emit_out(last=False):
                        if not pending:
                            return
                        i, a_s, pieces = pending.pop(0)
                        for pi, (q0, q1v) in enumerate(pieces):
                            st = not started[0]
                            started[0] = True
                            nc.tensor.matmul(
                                o_ps[:, q0:q1v], va[:, TB * i : TB * (i + 1)],
                                a_s[:, q0:q1v], start=st,
                                stop=(last and pi == len(pieces) - 1),
                            )

                    def note(entry):
                        # 2-strip lookahead: out(i) is issued after the score
                        # matmuls of strip i+2 (5 sps banks hold 2.5 strips)
                        if len(pending) >= 2:
                            emit_out()
                        pending.append(entry)

                    def full_strip(i):
                        """Scores + combine for an unmasked strip."""
                        a1, a2 = blk_scores(i, 0, SQ)
                        a_s = apool.tile([128, SQ], BF16, tag="a")
                        t = s1p.tile([128, SQ], F32, tag="s1")
                        nc.scalar.copy(t[:], a1[:])
                        nc.vector.tensor_mul(a_s[:], t[:], a2[:])
                        return a_s, [(0, 512)]

                    def sub_strip():
                        """The half-masked strip (keys 2c+1): only query cols
                        128:384 are live."""
                        i = 2 * c + 1
                        a1, a2 = blk_scores(i, 128, 384)
                        a_s = apool.tile([128, SQ], BF16, tag="a")
                        t = s1p.tile([128, 256], F32, tag="s1h")
                        nc.vector.tensor_mul(t[:], a1[:, 128:384], dmask_t[:])
                        nc.vector.tensor_mul(a_s[:, 128:384], t[:],
                                             a2[:, 128:384])
                        return i, a_s, [(128, 384)]

                    def diag_strip():
                        """Diag strip: mask only the edge cols
                        {0:128} u {384:512}."""
                        i = 2 * c
                        a1, a2 = blk_scores(i, 0, SQ)
                        t = s1p.tile([128, SQ], F32, tag="s1d")
                        dmv = dmask_t[:].rearrange("p (r n) -> p r n", r=2)
                        nc.vector.tensor_mul(
                            t[:, 0:256].rearrange("p (r n) -> p r n", r=2),
                            _edge(a1), dmv)
                        a_s = apool.tile([128, SQ], BF16, tag="a")
                        nc.vector.tensor_mul(
                            _edge(a_s),
                            t[:, 0:256].rearrange("p (r n) -> p r n", r=2),
                            _edge(a2))
                        nc.scalar.copy(t[:, 256:512], a1[:, 128:384])
                        nc.vector.tensor_mul(a_s[:, 128:384], t[:, 256:512],
                                             a2[:, 128:384])
                        return i, a_s, [(0, 512)]

                    def causal_steps():
                        if c == 0:
                            # chunk 0's sub strip runs full width with the
                            # wide mask to own the PSUM start
                            a1, a2 = blk_scores(1, 0, SQ)
                            a_s = apool.tile([128, SQ], BF16, tag="a")
                            t = s1p.tile([128, SQ], F32, tag="s1")
                            nc.vector.tensor_mul(t[:], a1[:], dmaskw_t[:])
                            nc.vector.tensor_mul(a_s[:], t[:], a2[:])
                            note((1, a_s, [(0, 512)]))
                            yield
                            note(diag_strip())
                            yield
                        else:
                            # order: full0 (owns PSUM start), sub, diag,
                            # remaining fulls (last full has the shortest
                            # combine chain -> minimal tail before epilogue)
                            a_s, pieces = full_strip(0)
                            note((0, a_s, pieces))
                            yield
                            note(sub_strip())
                            yield
                            note(diag_strip())
                            yield
                            for i in range(1, 2 * c):
                                a_s, pieces = full_strip(i)
                                note((i, a_s, pieces))
                                yield
                        # flush: last pending carries the stop flag
                        while len(pending) > 1:
                            emit_out()
                        emit_out(last=True)
                        yield

                    for _ in causal_steps():
                        fill()
                else:
                    for i in range(ni):
                        a1, a2 = blk_scores(i, 0, SQ)
                        a_s = apool.tile([128, SQ], BF16, tag="a")
                        if variant == "generic":
                            gm = gmp.tile([128, SQ], F32, tag="gm")
                            nc.sync.dma_start(gm[:], gmask.ap()[c, i])
                            t = s1p.tile([128, SQ], F32, tag="s1d")
                            nc.vector.tensor_mul(t[:], a1[:], gm[:])
                            nc.vector.tensor_mul(a_s[:], t[:], a2[:])
                        else:
                            t = s1p.tile([128, SQ], F32, tag="s1d")
                            nc.scalar.copy(t[:], a1[:])
                            nc.vector.tensor_mul(a_s[:], t[:], a2[:])
                        nc.tensor.matmul(
                            o_ps[:], va[:, TB * i : TB * (i + 1)], a_s[:],
                            start=(i == 0), stop=(i == ni - 1),
                        )
                        fill()
                if bw is not None:
                    for _ in bw:
                        pass
                # drain any remaining epilogue steps of the previous chunk
                if prev_epi is not None:
                    for _ in prev_epi:
                        pass
                prev_epi = epilogue(c, o_ps)
                # kick off the first steps (o_sb copy) right away
                next(prev_epi, None)

            for _ in prev_epi:
                pass

    nc.compile()
    return nc


def _perm_blocks(c, p):
    """Order of the 4 query blocks of chunk c: parity-p blocks first,
    then the other parity in descending order (mask-shape alignment)."""
    return [4 * c + p, 4 * c + 2 + p, 4 * c + 3 - p, 4 * c + 1 - p]


def _host_inputs(x, cos, sin, Wq1, Wq2, Wk1, Wk2, Wv, Wo, variant, mask):
    wimg = np.empty((128, WIMG_COLS), np.float32)
    for name, w in (("q1", Wq1), ("q2", Wq2), ("k1", Wk1), ("k2", Wk2),
                    ("v", Wv * (1.0 / DH))):
        off = WOFF[name]
        # wimg[p_, off + j*128 + dcol] = w[j*128 + p_, dcol]
        wimg[:, off : off + D] = (
            w.reshape(DC, 128, DH).transpose(1, 0, 2).reshape(128, D)
        )
    wimg[:, WO_OFF:] = Wo  # [128 d, D]
    wimg = wimg.astype(ml_dtypes.bfloat16)
    ident = np.eye(128, dtype=np.float32)

    # tri-keep: key g*128+t visible to query g*128+cc iff t <= cc
    tt = np.arange(128)[:, None]
    ccol = np.arange(128)[None, :]
    tri = (tt <= ccol).astype(np.float32)

    in_maps = []
    perms = []
    for core in range(8):
        b, p = divmod(core, 2)
        blocks = np.concatenate(
            [np.asarray(_perm_blocks(c, p)) for c in range(NCH)]
        )
        qperm = (blocks[:, None] * 128 + np.arange(128)[None, :]).reshape(-1)
        perms.append(qperm)
        xsel = x[b][qperm]  # [S, D]
        xq = np.ascontiguousarray(
            xsel.reshape(NCH, SQ, DC, 128).transpose(0, 3, 2, 1)
        ).reshape(NCH, 128, DC * SQ).astype(ml_dtypes.bfloat16)
        csarr = np.empty((NCH, 128, 2 * SQ), np.float32)
        cosl = cos[qperm].reshape(NCH, SQ, 64).transpose(0, 2, 1)
        sinl = sin[qperm].reshape(NCH, SQ, 64).transpose(0, 2, 1)
        csarr[:, 0:64, 0:SQ] = cosl
        csarr[:, 64:128, 0:SQ] = cosl
        # rope via partition swap: rot = [x2; x1], so the sin multiplier
        # is [-sin; sin] (negation folded into the table)
        csarr[:, 0:64, SQ:] = -sinl
        csarr[:, 64:128, SQ:] = sinl
        csarr = csarr.astype(ml_dtypes.bfloat16)
        m = {"xq": xq, "cs": csarr, "wimg": wimg, "iden": ident}
        if variant == "causal":
            X = np.full((128, 128), 1.0 - p, np.float32)
            m["dmask"] = np.concatenate([tri, X], axis=1)
            m["dmaskw"] = np.concatenate(
                [np.zeros((128, 128), np.float32), tri, X,
                 np.zeros((128, 128), np.float32)], axis=1)
        elif variant == "generic":
            # mult[s, t] = 0 where mask True; per core: [NCH, NSTRIP, 128, SQ]
            mult = (~mask).astype(np.float32)  # [S(q), S(k)]
            gm = np.empty((NCH, NSTRIP, 128, SQ), np.float32)
            for c in range(NCH):
                qcols = qperm[c * SQ : (c + 1) * SQ]
                sub = mult[qcols][:, :].T  # [S(k), SQ]
                for i in range(NSTRIP):
                    kb = 2 * i + p
                    gm[c, i] = sub[kb * 128 : (kb + 1) * 128, :]
            m["gmask"] = gm
        in_maps.append(m)
    return in_maps, perms


def kernel(x, cos, sin, causal_mask, Wq1, Wq2, Wk1, Wk2, Wv, Wo):
    x = np.ascontiguousarray(np.asarray(x, dtype=np.float32))
    cos = np.asarray(cos, dtype=np.float32)
    sin = np.asarray(sin, dtype=np.float32)
    mask = np.asarray(causal_mask, dtype=bool)
    args = [np.asarray(w, dtype=np.float32)
            for w in (Wq1, Wq2, Wk1, Wk2, Wv, Wo)]

    if not mask.any():
        variant = "dense"
    else:
        triu = np.triu(np.ones((S, S), dtype=bool), k=1)
        variant = "causal" if np.array_equal(mask, triu) else "generic"

    if variant not in _compiled:
        _compiled[variant] = _build(variant)
    nc = _compiled[variant]

    in_maps, perms = _host_inputs(x, cos, sin, *args, variant, mask)
    res = run_bass_kernel_spmd(nc, in_maps, list(range(8)))

    out = np.empty((B, S, D), np.float32)
    for b in range(B):
        acc = None
        for p in range(2):
            core = 2 * b + p
            yc = (np.asarray(res.results[core]["y"], dtype=np.float32)
                  .reshape(NCH, 128, SQ // 128, D)
                  .transpose(0, 2, 1, 3)
                  .reshape(S, D))
            inv = np.empty(S, np.int64)
            inv[perms[core]] = np.arange(S)
            contrib = yc[inv]
            acc = contrib if acc is None else acc + contrib
        out[b] = acc
    return out
